# revision 1
# baseline (speedup 1.0000x reference)
"""Farthest-point-sampling (npoint=2) Bass kernel for Trainium2.

Problem: xyz [1, 64, 3, 262144] fp32 -> indices [64, 2] (int64 on host).
Per batch b:
  idx0 = argmax_n y[n]            (y = coord plane 1)
  c    = (x,y,z)[idx0]
  idx1 = argmax_n ((x-cx)^2 + (y-cy)^2 + (z-cz)^2)
argmax = first occurrence on ties (jnp.argmax semantics).

Sharding: data-parallel over batch; 8 NeuronCores x 8 batches each.

Per-core structure (planes viewed as [128, 2048] fp32):
  Phase 0 (all 8 batches): DMA y plane; VectorE Max8 + MaxIndex ->
    per-partition (top-8, cols); stash col-0 max and (N - global_idx)
    candidate into defer tiles.
  Y finale (batched): PE-transpose the 8 batches' [128,1] pairs into
    rows, then reduce/select tiny ops produce idx0 per batch.
    candidate = N - global_idx so the max picks the smallest index among
    equal maxima (first-occurrence tie semantics).
  Per batch phase B: PE ones-matmul broadcasts idx0 -> [3,1]; offsets
    stt; indirect-DMA gather of centroid [3,1]; PE transpose + ScalarE
    negate -> [1,3]; PE ones-matmul broadcast -> [128,3] bias tile;
    paired x+z DMA; ScalarE Square(v + (-c)) x3; GpSimd adds
    s1 = sqx+sqy, s2 = s1+sqz; VectorE Max8 + MaxIndex on s2; stash.
  Dist finale (batched): same as Y finale -> idx1 per batch.
All GPSIMD ops are 'standard'-library (iota, tensor_tensor) or DGE —
no mid-kernel ucode library swaps.
"""

import numpy as np

import concourse.bacc as bacc
import concourse.bass as bass
import concourse.mybir as mybir
from concourse.masks import make_identity
from concourse.tile import TileContext

B = 64  # full batch
N_CORES = 8
BPC = B // N_CORES  # batches per core
N = 262144
P = 128
COLS = N // P  # 2048
BIGK = float(N)

F32 = mybir.dt.float32
U32 = mybir.dt.uint32
I32 = mybir.dt.int32
AX = mybir.AxisListType.X
OP = mybir.AluOpType
SQUARE = mybir.ActivationFunctionType.Square


def build_nc():
    nc = bacc.Bacc()
    xin = nc.dram_tensor("xyz", [BPC, 3, N], F32, kind="ExternalInput")
    out = nc.dram_tensor("idx", [1, 2 * BPC], I32, kind="ExternalOutput")

    with TileContext(nc) as tc:
        with (
            tc.tile_pool(name="consts", bufs=1) as consts,
            tc.tile_pool(name="ypool", bufs=BPC) as ypool,
            tc.tile_pool(name="big", bufs=2) as big,
            tc.tile_pool(name="small", bufs=4) as small,
            tc.tile_pool(name="acc", bufs=1) as acc,
            tc.tile_pool(name="psb", bufs=2, space="PSUM") as psb,
            tc.tile_pool(name="psf", bufs=1, space="PSUM") as psf,
        ):
            # ---- constants ----
            ident = consts.tile([P, P], F32)
            make_identity(nc, ident)
            ones = consts.tile([1, P], F32)
            nc.vector.memset(ones, 1.0)
            # revb[p] = N - p*COLS ; pbase[c] = c*N   (exact in f32 < 2^24)
            revb_i = consts.tile([P, 1], I32)
            nc.gpsimd.iota(revb_i, pattern=[[0, 1]], base=N, channel_multiplier=-COLS)
            revb_f = consts.tile([P, 1], F32)
            nc.vector.tensor_copy(revb_f, revb_i)
            pbase = consts.tile([3, 1], I32)
            nc.gpsimd.iota(pbase, pattern=[[0, 1]], base=0, channel_multiplier=N)

            out_i = acc.tile([1, 2 * BPC], I32)  # cols 0..7 idx0, 8..15 idx1
            # wide defer tiles: max8/max_index write straight into them
            dYV8 = acc.tile([P, 8 * BPC], F32)
            dYI8 = acc.tile([P, 8 * BPC], U32)
            dDV8 = acc.tile([P, 8 * BPC], F32)
            dDI8 = acc.tile([P, 8 * BPC], U32)

            def col0(t):
                return t.rearrange("p (b k) -> p b k", k=8)[:, :, 0]

            def batched_finale(dv8, di8, out_cols, tagp):
                """dv8/di8: [P, 8*BPC] per-batch top-8 (vals, cols).
                Returns SBUF [1, BPC] f32 of winning global indices; also
                writes them (cast i32) into out_i[:, out_cols]."""
                i8f = small.tile([P, BPC], F32, tag=f"i8f{tagp}")
                nc.vector.tensor_copy(i8f, col0(di8))
                candall = small.tile([P, BPC], F32, tag=f"ca{tagp}")
                nc.vector.tensor_sub(
                    candall, revb_f.to_broadcast([P, BPC]), i8f
                )
                ptv = psf.tile([BPC, 2 * P], F32, tag="ptv")
                nc.tensor.transpose(ptv[0:BPC, 0:P], col0(dv8), ident)
                nc.tensor.transpose(ptv[0:BPC, P : 2 * P], candall, ident)
                rows = small.tile([BPC, 2 * P], F32, tag=f"rows{tagp}")
                nc.vector.tensor_copy(rows, ptv)
                mxs = small.tile([BPC, 1], F32, tag=f"mxs{tagp}")
                nc.vector.tensor_reduce(mxs, rows[:, 0:P], axis=AX, op=OP.max)
                cands = small.tile([BPC, P], F32, tag=f"cands{tagp}")
                nc.vector.scalar_tensor_tensor(
                    out=cands,
                    in0=rows[:, 0:P],
                    scalar=mxs[:, 0:1],
                    in1=rows[:, P : 2 * P],
                    op0=OP.is_equal,
                    op1=OP.mult,
                )
                rs = small.tile([BPC, 1], F32, tag=f"rs{tagp}")
                nc.vector.tensor_reduce(rs, cands, axis=AX, op=OP.max)
                idxs = small.tile([BPC, 1], F32, tag=f"idxs{tagp}")
                nc.vector.tensor_scalar(
                    out=idxs, in0=rs, scalar1=-1.0, scalar2=BIGK,
                    op0=OP.mult, op1=OP.add,
                )
                pti = psf.tile([1, BPC], F32, tag="pti")
                nc.tensor.transpose(pti, idxs, ident[0:BPC, 0:BPC])
                rowi = small.tile([1, BPC], F32, tag=f"rowi{tagp}")
                nc.vector.tensor_copy(rowi, pti)
                nc.scalar.copy(out_i[0:1, out_cols], rowi)
                return rowi

            # ---------- phase 0: y argmax per batch ----------
            tys = []
            for b in range(BPC):
                ty = ypool.tile([P, COLS], F32, tag="ty")
                tys.append(ty)
                nc.sync.dma_start(ty, xin[b, 1].rearrange("(p m) -> p m", p=P))
                nc.vector.max(out=dYV8[:, 8 * b : 8 * b + 8], in_=ty)
                nc.vector.max_index(
                    dYI8[:, 8 * b : 8 * b + 8], dYV8[:, 8 * b : 8 * b + 8], ty
                )

            idx0row = batched_finale(dYV8, dYI8, slice(0, BPC), "y")

            # ---------- phase B per batch ----------
            for b in range(BPC):
                # idx0 -> [3,1] via PE ones-matmul; offsets; gather centroid
                p3 = psb.tile([3, 1], F32, tag="p3")
                nc.tensor.matmul(
                    p3, ones[0:1, 0:3], idx0row[0:1, b : b + 1],
                    start=True, stop=True,
                )
                offs = small.tile([3, 1], U32, tag="offs")
                # offs[c] = idx0 + b*3N + c*N (flat index into xin)
                nc.vector.scalar_tensor_tensor(
                    out=offs, in0=p3, scalar=float(b * 3 * N), in1=pbase,
                    op0=OP.add, op1=OP.add,
                )
                c3 = small.tile([3, 1], F32, tag="c3")
                nc.gpsimd.indirect_dma_start(
                    out=c3,
                    out_offset=None,
                    in_=xin.rearrange("b c n -> (b c n)")[:, None],
                    in_offset=bass.IndirectOffsetOnAxis(ap=offs[0:3, 0:1], axis=0),
                )
                # negate + broadcast to [128,3] bias tile via PE
                pc3 = psb.tile([1, 3], F32, tag="pc3")
                nc.tensor.transpose(pc3, c3, ident[0:3, 0:3])
                negrow = small.tile([1, 3], F32, tag="negrow")
                nc.scalar.mul(negrow, pc3, -1.0)
                pnegc = psb.tile([P, 3], F32, tag="pnegc")
                nc.tensor.matmul(pnegc, ones, negrow, start=True, stop=True)
                negc = small.tile([P, 3], F32, tag="negc")
                nc.vector.tensor_copy(negc, pnegc)

                # x and z planes in one strided DMA: [P, 2, COLS]
                txz = big.tile([P, 2, COLS], F32, tag="txz")
                nc.sync.dma_start(
                    txz,
                    xin[b, 0::2].rearrange("c (p m) -> p c m", p=P),
                )
                sqx = big.tile([P, COLS], F32, tag="sqx")
                nc.scalar.activation(sqx, txz[:, 0], SQUARE, bias=negc[:, 0:1])
                sqy = big.tile([P, COLS], F32, tag="sqy")
                nc.scalar.activation(sqy, tys[b], SQUARE, bias=negc[:, 1:2])
                sqz = big.tile([P, COLS], F32, tag="sqz")
                nc.scalar.activation(sqz, txz[:, 1], SQUARE, bias=negc[:, 2:3])

                # adds split ~75/25 between GpSimd and VectorE
                CS = 1536
                s1 = big.tile([P, COLS], F32, tag="s1")
                nc.gpsimd.tensor_add(s1[:, 0:CS], sqx[:, 0:CS], sqy[:, 0:CS])
                nc.vector.tensor_add(s1[:, CS:], sqx[:, CS:], sqy[:, CS:])
                s2 = big.tile([P, COLS], F32, tag="s2")
                nc.gpsimd.tensor_add(s2[:, 0:CS], s1[:, 0:CS], sqz[:, 0:CS])
                nc.vector.tensor_add(s2[:, CS:], s1[:, CS:], sqz[:, CS:])

                nc.vector.max(out=dDV8[:, 8 * b : 8 * b + 8], in_=s2)
                nc.vector.max_index(
                    dDI8[:, 8 * b : 8 * b + 8], dDV8[:, 8 * b : 8 * b + 8], s2
                )

            batched_finale(dDV8, dDI8, slice(BPC, 2 * BPC), "d")

            nc.sync.dma_start(out[:, :], out_i[:, :])

    nc.compile()
    return nc


_NC_CACHE = None


def _get_nc():
    global _NC_CACHE
    if _NC_CACHE is None:
        _NC_CACHE = build_nc()
    return _NC_CACHE


def kernel(xyz: np.ndarray) -> np.ndarray:
    from concourse.bass_utils import run_bass_kernel_spmd

    assert xyz.shape == (1, B, 3, N), xyz.shape
    xyz = np.ascontiguousarray(xyz, dtype=np.float32)
    nc = _get_nc()
    in_maps = [
        {"xyz": np.ascontiguousarray(xyz[0, k * BPC : (k + 1) * BPC])}
        for k in range(N_CORES)
    ]
    res = run_bass_kernel_spmd(nc, in_maps, core_ids=list(range(N_CORES)))
    # out layout per core: [1, 16] = [idx0 x8 | idx1 x8]
    outs = [res.results[k]["idx"].reshape(2, BPC).T for k in range(N_CORES)]
    return np.concatenate(outs, axis=0).astype(np.int64)

